# revision 6
# baseline (speedup 1.0000x reference)
"""MultiHeadLTC Trainium2 kernel.

V=8 independent LTC heads -> one head per NeuronCore (expert/model parallel).
Per core: B=512, T=64 timesteps x 6 implicit-ODE unfolds, U=64 units.

Device layout (per core):
  state vT [65, 512] SBUF : rows 0..63 = v[u, b], row 64 = ones (bias row).
  Per unfold:
    - 2 init matmuls seed the PSUM accumulator with
        rows 0..63  : cm_t*v + gl*vleak + wnum_sensory   (numerator base)
        rows 64..127: cm_t + gl + eps + wden_sensory     (denominator base)
    - 32 "z" matmuls  z_q[p,b] = sigma_f*(v[i(f),b] - mu_f), f = 128q+p
      (lhsT A_q [65,128] carries sigma-scaled selector rows + bias row)
    - Sigmoid on ScalarE over big PSUM chunks (4/3-bank chunks)
    - 32 accumulating "reduce" matmuls with block-sparse weight maps M_q
      add  sum_i w_p*erev*sig(..)  to numerator rows and
           sum_i w_p*sig(..)      to denominator rows
    - v <- num * reciprocal_approx(den)   (2 DVE ops)
Final: feats[u,b] = v*output_w + output_b  -> DMA out; classifier done on host
(67 MFLOP, 0.003% of total work).
"""

import os
from contextlib import ExitStack

import numpy as np

UNFOLDS, EPS = 6, 1e-8
V, B, T, I, U, H, C = 8, 512, 64, 1, 64, 256, 10
NQ = (U * U) // 128  # 32 z/reduce matmul chunks per unfold
CHUNK_Q = [4, 3, 4, 3, 4, 3, 4, 3, 4]  # q's per ACT chunk (sum = 32)


def _softplus(x):
    return np.logaddexp(x.astype(np.float64), 0.0)


def prep_core(inp, v):
    """Host-side precompute of per-core device inputs (all float32)."""
    g = {k: np.asarray(inp[k])[v].astype(np.float64) for k in
         ("gleak", "vleak", "cm", "w", "sigma", "mu", "erev",
          "sensory_w", "sensory_sigma", "sensory_mu", "sensory_erev",
          "input_w", "input_b", "output_w", "output_b")}
    x = np.asarray(inp["x"])[v].astype(np.float32)  # [B, T, I]
    cm_t = _softplus(g["cm"]) * UNFOLDS
    gl = _softplus(g["gleak"])
    w_p = _softplus(g["w"])
    sw_p = _softplus(g["sensory_w"])
    sigma, mu, erev = g["sigma"], g["mu"], g["erev"]
    ssig, smu, serev = (g["sensory_sigma"][0], g["sensory_mu"][0],
                        g["sensory_erev"][0])
    iw, ib = g["input_w"][0], g["input_b"][0]

    f = np.arange(U * U)
    i_f, j_f = f // U, f % U
    sig_f, mu_f = sigma[i_f, j_f], mu[i_f, j_f]
    A = np.zeros((U + 1, U * U))
    A[i_f, f] = sig_f
    A[U, f] = -sig_f * mu_f
    Aall = A.reshape(U + 1, NQ, 128)                  # [65, q, p]

    we = w_p * erev
    M = np.zeros((U * U, 2 * U))
    M[f, j_f] = we[i_f, j_f]
    M[f, U + j_f] = w_p[i_f, j_f]
    Mall = np.ascontiguousarray(
        M.reshape(NQ, 128, 2 * U).transpose(1, 0, 2))  # [p, q, m]

    cmv = np.zeros((U, 128))
    cmv[np.arange(U), np.arange(U)] = cm_t
    ident = np.eye(128)

    Asrow = (ssig * iw)[None, :]                      # [1, U]
    cvec = np.stack([
        sw_p[0] * serev,                              # 0: cne
        sw_p[0],                                      # 1: cnd
        gl * g["vleak"],                              # 2: glv
        cm_t + gl + EPS,                              # 3: cden
        ssig * (ib - smu),                            # 4: sensory ACT bias
        g["output_w"],                                # 5: ow
        g["output_b"],                                # 6: ob
        np.zeros(U),                                  # 7: pad
    ], axis=1)                                        # [U, 8]
    xT = np.ascontiguousarray(x[:, :, 0].T)             # [T, B]

    f32 = np.float32
    return dict(xT=xT.astype(f32), Aall=Aall.astype(f32),
                Mall=Mall.astype(f32), cmv=cmv.astype(f32),
                ident=ident.astype(f32), Asrow=Asrow.astype(f32),
                cvec=cvec.astype(f32))


def build_nc(nsteps=T):
    import concourse.tile as tile
    from concourse import bacc, mybir

    f32 = mybir.dt.float32
    AF = mybir.ActivationFunctionType
    OP = mybir.AluOpType

    nc = bacc.Bacc("TRN2", target_bir_lowering=False)
    xT_d = nc.dram_tensor("xT", [T, B], f32, kind="ExternalInput")
    Aall_d = nc.dram_tensor("Aall", [U + 1, NQ, 128], f32, kind="ExternalInput")
    Mall_d = nc.dram_tensor("Mall", [128, NQ, 128], f32, kind="ExternalInput")
    cmv_d = nc.dram_tensor("cmv", [U, 128], f32, kind="ExternalInput")
    ident_d = nc.dram_tensor("ident", [128, 128], f32, kind="ExternalInput")
    Asrow_d = nc.dram_tensor("Asrow", [1, U], f32, kind="ExternalInput")
    cvec_d = nc.dram_tensor("cvec", [U, 8], f32, kind="ExternalInput")
    feats_d = nc.dram_tensor("feats", [U, B], f32, kind="ExternalOutput")

    with tile.TileContext(nc) as tc, ExitStack() as ctx:
        const = ctx.enter_context(tc.tile_pool(name="const", bufs=1))
        sp = ctx.enter_context(tc.tile_pool(name="sp", bufs=2))
        pz = ctx.enter_context(tc.tile_pool(name="pz", bufs=1, space="PSUM"))

        Aall_sb = const.tile([U + 1, NQ, 128], f32)
        nc.sync.dma_start(out=Aall_sb, in_=Aall_d[:, :, :])
        Mall_sb = const.tile([128, NQ, 128], f32)
        nc.sync.dma_start(out=Mall_sb, in_=Mall_d[:, :, :])
        cmv_sb = const.tile([U, 128], f32)
        nc.sync.dma_start(out=cmv_sb, in_=cmv_d[:, :])
        ident_sb = const.tile([128, 128], f32)
        nc.sync.dma_start(out=ident_sb, in_=ident_d[:, :])
        Asrow_sb = const.tile([1, U], f32)
        nc.sync.dma_start(out=Asrow_sb, in_=Asrow_d[:, :])
        cvec_sb = const.tile([U, 8], f32)
        nc.sync.dma_start(out=cvec_sb, in_=cvec_d[:, :])

        vT = const.tile([U + 1, B], f32)
        nc.vector.memset(vT[0:U, :], 0.0)
        nc.vector.memset(vT[U:U + 1, :], 1.0)

        for t in range(nsteps):
            # ---- sensory path (per step) ----
            xrow = sp.tile([1, B], f32, tag="xrow")
            nc.sync.dma_start(out=xrow[:, :], in_=xT_d[t:t + 1, :])
            zs = pz.tile([128, B], f32, tag="zA")
            nc.tensor.matmul(zs[0:U, :], Asrow_sb[:, :], xrow[0:1, :],
                             start=True, stop=True)
            sact = sp.tile([U, B], f32, tag="sact")
            nc.scalar.activation(sact[:, :], zs[0:U, :], AF.Sigmoid,
                                 bias=cvec_sb[:, 4:5], scale=1.0)
            bb = sp.tile([128, B], f32, tag="bb")
            nc.vector.tensor_scalar(bb[0:U, :], sact[:, :],
                                    cvec_sb[:, 0:1], cvec_sb[:, 2:3],
                                    OP.mult, OP.add)
            nc.vector.tensor_scalar(bb[U:128, :], sact[:, :],
                                    cvec_sb[:, 1:2], cvec_sb[:, 3:4],
                                    OP.mult, OP.add)

            for k in range(UNFOLDS):
                acc = pz.tile([128, B], f32, tag="acc")
                nc.tensor.matmul(acc[:, :], ident_sb[:, :], bb[:, :],
                                 start=True, stop=False)
                nc.tensor.matmul(acc[:, :], cmv_sb[:, :], vT[0:U, :],
                                 start=False, stop=False, skip_group_check=True)
                qi = 0
                for nq in CHUNK_Q:
                    fd = nq * 512
                    tag = "zA" if nq == 4 else "zB"
                    z = pz.tile([128, fd], f32, tag=tag)
                    for j in range(nq):
                        nc.tensor.matmul(z[:, j * 512:(j + 1) * 512],
                                         Aall_sb[:, qi + j, :], vT[:, :],
                                         start=True, stop=True)
                    S = sp.tile([128, fd], f32, tag="S" + tag[1])
                    nc.scalar.activation(S[:, :], z[:, :], AF.Sigmoid)
                    for j in range(nq):
                        nc.tensor.matmul(acc[:, :], Mall_sb[:, qi + j, :],
                                         S[:, j * 512:(j + 1) * 512],
                                         start=False, stop=(qi + j == NQ - 1),
                                         skip_group_check=True)
                    qi += nq
                den_cp = sp.tile([U, B], f32, tag="den_cp")
                nc.vector.tensor_copy(den_cp[:, :], acc[U:128, :])
                rec = sp.tile([U, B], f32, tag="rec")
                nc.vector.reciprocal_approx_fast(out=rec[:, :],
                                                 in_=den_cp[:, :])
                nc.vector.tensor_tensor(vT[0:U, :], acc[0:U, :], rec[:, :],
                                        OP.mult)

        outsb = sp.tile([U, B], f32, tag="outsb")
        nc.vector.tensor_scalar(outsb[:, :], vT[0:U, :],
                                cvec_sb[:, 5:6], cvec_sb[:, 6:7],
                                OP.mult, OP.add)
        nc.sync.dma_start(out=feats_d[:, :], in_=outsb[:, :])
    nc.compile()
    return nc


_NC_CACHE = {}


def _get_nc(nsteps=T):
    if nsteps not in _NC_CACHE:
        _NC_CACHE[nsteps] = build_nc(nsteps)
    return _NC_CACHE[nsteps]


def run_cores(inputs, trace=False, nsteps=T):
    """Run the 8-core SPMD kernel; returns (list of per-core feats [U,B], perf)."""
    from concourse.bass_utils import run_bass_kernel_spmd

    nc = _get_nc(nsteps)
    in_maps = [prep_core(inputs, v) for v in range(V)]
    res = run_bass_kernel_spmd(nc, in_maps, core_ids=list(range(V)),
                               trace=trace)
    return [r["feats"] for r in res.results], res


def kernel(**inputs) -> np.ndarray:
    feats_list, _ = run_cores(inputs, trace=False)
    feats = np.zeros((B, V * U), dtype=np.float32)
    for v in range(V):
        feats[:, v * U:(v + 1) * U] = feats_list[v].T
    W1 = np.asarray(inputs["W1"], dtype=np.float32)
    b1 = np.asarray(inputs["b1"], dtype=np.float32)
    W2 = np.asarray(inputs["W2"], dtype=np.float32)
    b2 = np.asarray(inputs["b2"], dtype=np.float32)
    h = np.maximum(feats @ W1 + b1, 0.0)
    return (h @ W2 + b2).astype(np.float32)


# revision 16
# speedup vs baseline: 291.7560x; 291.7560x over previous
"""MultiHeadLTC Trainium2 kernel.

V=8 independent LTC heads -> one head per NeuronCore (expert/model parallel).
Per core: B=512, T=64 timesteps x 6 implicit-ODE unfolds, U=64 units.

Device layout (per core):
  state vT [65, 512] SBUF : rows 0..63 = v[u, b], row 64 = ones (bias row).
  Per unfold:
    - 2 init matmuls seed the PSUM accumulator with
        rows 0..63  : cm_t*v + gl*vleak + wnum_sensory   (numerator base)
        rows 64..127: cm_t + gl + eps + wden_sensory     (denominator base)
    - 32 "z" matmuls  z_q[p,b] = sigma_f*(v[i(f),b] - mu_f), f = 128q+p
      (lhsT A_q [65,128] carries sigma-scaled selector rows + bias row)
    - Sigmoid on ScalarE over big PSUM chunks (4/3-bank chunks)
    - 32 accumulating "reduce" matmuls with block-sparse weight maps M_q
      add  sum_i w_p*erev*sig(..)  to numerator rows and
           sum_i w_p*sig(..)      to denominator rows
    - v <- num * reciprocal_approx(den)   (2 DVE ops)
Final: feats[u,b] = v*output_w + output_b  -> DMA out; classifier done on host
(67 MFLOP, 0.003% of total work).
"""

import os
from contextlib import ExitStack

import ml_dtypes
import numpy as np

UNFOLDS, EPS = 6, 1e-8
V, B, T, I, U, H, C = 8, 512, 64, 1, 64, 256, 10
NQ = (U * U) // 128  # 32 z/reduce matmul chunks per unfold
CHUNK_Q = [4, 3, 4, 3, 4, 3, 4, 3, 4]  # q's per ACT chunk (sum = 32)


def _softplus(x):
    return np.logaddexp(x.astype(np.float64), 0.0)


def prep_core(inp, v):
    """Host-side precompute of per-core device inputs (all float32)."""
    g = {k: np.asarray(inp[k])[v].astype(np.float64) for k in
         ("gleak", "vleak", "cm", "w", "sigma", "mu", "erev",
          "sensory_w", "sensory_sigma", "sensory_mu", "sensory_erev",
          "input_w", "input_b", "output_w", "output_b")}
    x = np.asarray(inp["x"])[v].astype(np.float32)  # [B, T, I]
    cm_t = _softplus(g["cm"]) * UNFOLDS
    gl = _softplus(g["gleak"])
    w_p = _softplus(g["w"])
    sw_p = _softplus(g["sensory_w"])
    sigma, mu, erev = g["sigma"], g["mu"], g["erev"]
    ssig, smu, serev = (g["sensory_sigma"][0], g["sensory_mu"][0],
                        g["sensory_erev"][0])
    iw, ib = g["input_w"][0], g["input_b"][0]

    f = np.arange(U * U)
    i_f, j_f = f // U, f % U
    sig_f, mu_f = sigma[i_f, j_f], mu[i_f, j_f]
    A = np.zeros((U + 1, U * U))
    A[i_f, f] = sig_f
    A[U, f] = -sig_f * mu_f
    Aall = A.reshape(U + 1, NQ, 128)                  # [65, q, p]

    we = w_p * erev
    M = np.zeros((U * U, 2 * U))
    M[f, j_f] = we[i_f, j_f]
    M[f, U + j_f] = w_p[i_f, j_f]
    Mall = np.ascontiguousarray(
        M.reshape(NQ, 128, 2 * U).transpose(1, 0, 2))  # [p, q, m]

    cmv = np.zeros((U, 128))
    cmv[np.arange(U), np.arange(U)] = cm_t
    ident = np.eye(128)

    Asrow = (ssig * iw)[None, :]                      # [1, U]
    cvec = np.stack([
        sw_p[0] * serev,                              # 0: cne
        sw_p[0],                                      # 1: cnd
        gl * g["vleak"],                              # 2: glv
        cm_t + gl + EPS,                              # 3: cden
        ssig * (ib - smu),                            # 4: sensory ACT bias
        g["output_w"],                                # 5: ow
        g["output_b"],                                # 6: ob
        np.zeros(U),                                  # 7: pad
    ], axis=1)                                        # [U, 8]
    xT = np.ascontiguousarray(x[:, :, 0].T)             # [T, B]

    f32 = np.float32
    bf16 = ml_dtypes.bfloat16
    return dict(xT=xT.astype(f32), Aall=Aall.astype(bf16),
                Mall=Mall.astype(bf16), cmv=cmv.astype(f32),
                ident=ident.astype(f32), Asrow=Asrow.astype(f32),
                cvec=cvec.astype(f32))


def build_nc(nsteps=T, reps=1):
    import concourse.tile as tile
    from concourse import bacc, mybir

    f32 = mybir.dt.float32
    bf16 = mybir.dt.bfloat16
    AF = mybir.ActivationFunctionType
    OP = mybir.AluOpType

    nc = bacc.Bacc("TRN2", target_bir_lowering=False)
    xT_d = nc.dram_tensor("xT", [T, B], f32, kind="ExternalInput")
    Aall_d = nc.dram_tensor("Aall", [U + 1, NQ, 128], bf16,
                            kind="ExternalInput")
    Mall_d = nc.dram_tensor("Mall", [128, NQ, 128], bf16,
                            kind="ExternalInput")
    cmv_d = nc.dram_tensor("cmv", [U, 128], f32, kind="ExternalInput")
    ident_d = nc.dram_tensor("ident", [128, 128], f32, kind="ExternalInput")
    Asrow_d = nc.dram_tensor("Asrow", [1, U], f32, kind="ExternalInput")
    cvec_d = nc.dram_tensor("cvec", [U, 8], f32, kind="ExternalInput")
    feats_d = nc.dram_tensor("feats", [U, B], f32, kind="ExternalOutput")

    with tile.TileContext(nc) as tc, ExitStack() as ctx:
        const = ctx.enter_context(tc.tile_pool(name="const", bufs=1))
        sp = ctx.enter_context(tc.tile_pool(name="sp", bufs=2))
        pz = ctx.enter_context(tc.tile_pool(name="pz", bufs=1, space="PSUM"))

        Aall_sb = const.tile([U + 1, NQ, 128], bf16)
        nc.sync.dma_start(out=Aall_sb, in_=Aall_d[:, :, :])
        Mall_sb = const.tile([128, NQ, 128], bf16)
        nc.sync.dma_start(out=Mall_sb, in_=Mall_d[:, :, :])
        cmv_sb = const.tile([U, 128], f32)
        nc.sync.dma_start(out=cmv_sb, in_=cmv_d[:, :])
        ident_sb = const.tile([128, 128], f32)
        nc.sync.dma_start(out=ident_sb, in_=ident_d[:, :])
        Asrow_sb = const.tile([1, U], f32)
        nc.sync.dma_start(out=Asrow_sb, in_=Asrow_d[:, :])
        cvec_sb = const.tile([U, 8], f32)
        nc.sync.dma_start(out=cvec_sb, in_=cvec_d[:, :])

        vT = const.tile([U + 1, B], f32)
        nc.vector.memset(vT[0:U, :], 0.0)
        nc.vector.memset(vT[U:U + 1, :], 1.0)
        vTb = const.tile([U + 1, B], bf16)   # bf16 shadow for z-matmul rhs
        nc.vector.memset(vTb[0:U, :], 0.0)
        nc.vector.memset(vTb[U:U + 1, :], 1.0)

        HB = B // 2                      # half-batch columns
        CQ = 4                           # q's per ACT chunk per half
        NCH = NQ // CQ                   # 8 chunks per half per unfold
        hs = [slice(0, HB), slice(HB, B)]

        for _rep in range(reps):
          for t in range(nsteps):
            # ---- sensory path (per step) ----
            xrow = sp.tile([1, B], f32, tag="xrow")
            nc.sync.dma_start(out=xrow[:, :], in_=xT_d[t:t + 1, :])
            zs = pz.tile([128, B], f32, tag="z0")
            nc.tensor.matmul(zs[0:U, :], Asrow_sb[:, :], xrow[0:1, :],
                             start=True, stop=True)
            sact = sp.tile([U, B], f32, tag="sact")
            nc.scalar.activation(sact[:, :], zs[0:U, :], AF.Sigmoid,
                                 bias=cvec_sb[:, 4:5], scale=1.0)
            bb = sp.tile([128, B], f32, tag="bb")
            nc.vector.tensor_scalar(bb[0:U, :], sact[:, :],
                                    cvec_sb[:, 0:1], cvec_sb[:, 2:3],
                                    OP.mult, OP.add)
            nc.vector.tensor_scalar(bb[U:128, :], sact[:, :],
                                    cvec_sb[:, 1:2], cvec_sb[:, 3:4],
                                    OP.mult, OP.add)

            for k in range(UNFOLDS):
                acc = [pz.tile([128, HB], f32, tag=f"acc{h}",
                               name=f"acc{h}_{t}_{k}")
                       for h in (0, 1)]
                for h in (0, 1):
                    nc.tensor.matmul(acc[h][:, :], ident_sb[:, :],
                                     bb[:, hs[h]], start=True, stop=False)
                    nc.tensor.matmul(acc[h][:, :], cmv_sb[:, :],
                                     vT[0:U, hs[h]], start=False, stop=False,
                                     skip_group_check=True)
                fd = CQ * HB
                seq = [(h, c) for c in range(NCH) for h in (0, 1)]
                pend = []

                def emit_red(h, c, S):
                    for j in range(CQ):
                        q = c * CQ + j
                        nc.tensor.matmul(acc[h][:, :], Mall_sb[:, q, :],
                                         S[:, j * HB:(j + 1) * HB],
                                         start=False, stop=(q == NQ - 1),
                                         skip_group_check=True)

                for idx, (h, c) in enumerate(seq):
                    slot = idx % 3
                    z = pz.tile([128, fd], f32, tag=f"z{slot}",
                                name=f"z_{t}_{k}_{idx}")
                    for j in range(CQ):
                        q = c * CQ + j
                        nc.tensor.matmul(z[:, j * HB:(j + 1) * HB],
                                         Aall_sb[:, q, :], vTb[:, hs[h]],
                                         start=True, stop=True)
                    S = sp.tile([128, fd], bf16, tag=f"S{slot}", bufs=2,
                                name=f"S_{t}_{k}_{idx}")
                    nc.scalar.activation(S[:, :], z[:, :], AF.Sigmoid)
                    pend.append((h, c, S))
                    if idx >= 2:
                        emit_red(*pend.pop(0))
                for item in pend:
                    emit_red(*item)
                for h in (0, 1):
                    den_cp = sp.tile([U, HB], f32, tag=f"den_cp{h}")
                    nc.vector.tensor_copy(den_cp[:, :], acc[h][U:128, :])
                    rec = sp.tile([U, HB], f32, tag=f"rec{h}")
                    nc.vector.reciprocal_approx_fast(out=rec[:, :],
                                                     in_=den_cp[:, :])
                    nc.vector.tensor_tensor(vT[0:U, hs[h]], acc[h][0:U, :],
                                            rec[:, :], OP.mult)
                    nc.vector.tensor_copy(vTb[0:U, hs[h]], vT[0:U, hs[h]])

        outsb = sp.tile([U, B], f32, tag="outsb")
        nc.vector.tensor_scalar(outsb[:, :], vT[0:U, :],
                                cvec_sb[:, 5:6], cvec_sb[:, 6:7],
                                OP.mult, OP.add)
        nc.sync.dma_start(out=feats_d[:, :], in_=outsb[:, :])
    nc.compile()
    return nc



_NC_CACHE = {}


def _get_nc(nsteps=T, reps=1):
    key = (nsteps, reps)
    if key not in _NC_CACHE:
        _NC_CACHE[key] = build_nc(nsteps, reps)
    return _NC_CACHE[key]


class CachedRunner:
    def __init__(self, nc, n_cores):
        import jax
        from jax.sharding import Mesh, PartitionSpec
        from jax.experimental.shard_map import shard_map
        from concourse import mybir
        from concourse.bass2jax import (_bass_exec_p, install_neuronx_cc_hook,
                                        partition_id_tensor)

        install_neuronx_cc_hook()
        self.nc = nc
        self.n_cores = n_cores
        partition_name = (nc.partition_id_tensor.name
                          if nc.partition_id_tensor else None)
        in_names, out_names, out_avals, zero_outs = [], [], [], []
        for alloc in nc.m.functions[0].allocations:
            if not isinstance(alloc, mybir.MemoryLocationSet):
                continue
            name = alloc.memorylocations[0].name
            if alloc.kind == "ExternalInput":
                if name != partition_name:
                    in_names.append(name)
            elif alloc.kind == "ExternalOutput":
                shape = tuple(alloc.tensor_shape)
                dtype = mybir.dt.np(alloc.dtype)
                out_names.append(name)
                out_avals.append(jax.core.ShapedArray(shape, dtype))
                zero_outs.append(np.zeros(shape, dtype))
        self.in_names, self.out_names = in_names, out_names
        self.out_avals, self.zero_outs = out_avals, zero_outs
        n_params, n_outs = len(in_names), len(out_names)
        self.n_params = n_params
        all_in = list(in_names) + list(out_names)
        if partition_name is not None:
            all_in.append(partition_name)

        def _body(*args):
            operands = list(args)
            if partition_name is not None:
                operands.append(partition_id_tensor())
            return tuple(_bass_exec_p.bind(
                *operands,
                out_avals=tuple(out_avals),
                in_names=tuple(all_in),
                out_names=tuple(out_names),
                lowering_input_output_aliases=(),
                sim_require_finite=True,
                sim_require_nnan=True,
                nc=nc,
            ))

        devices = jax.devices()[:n_cores]
        self.mesh = Mesh(np.asarray(devices), ("core",))
        in_specs = (PartitionSpec("core"),) * (n_params + n_outs)
        out_specs = (PartitionSpec("core"),) * n_outs
        # NOTE: no donation — lets us reuse the same zero buffers across calls.
        self.fn = jax.jit(shard_map(_body, mesh=self.mesh, in_specs=in_specs,
                                    out_specs=out_specs, check_rep=False),
                          keep_unused=True)
        self._jax = jax
        self._zeros_dev = None

    def put_inputs(self, in_maps):
        """Concatenate per-core inputs and move to devices; returns handle."""
        jax = self._jax
        from jax.sharding import NamedSharding, PartitionSpec
        concat_in = [
            np.concatenate([np.asarray(in_maps[c][name])
                            for c in range(self.n_cores)], axis=0)
            for name in self.in_names
        ]
        concat_zeros = [
            np.zeros((self.n_cores * z.shape[0], *z.shape[1:]), z.dtype)
            for z in self.zero_outs
        ]
        sh = NamedSharding(self.mesh, PartitionSpec("core"))
        args = [jax.device_put(a, sh) for a in concat_in + concat_zeros]
        jax.block_until_ready(args)
        return args

    def execute(self, args):
        out = self.fn(*args)
        self._jax.block_until_ready(out)
        return out

    def run(self, in_maps):
        """Full path: transfer + execute + fetch. Returns per-core dicts."""
        args = self.put_inputs(in_maps)
        out_arrs = self.execute(args)
        res = []
        for c in range(self.n_cores):
            res.append({
                name: np.asarray(out_arrs[i]).reshape(
                    self.n_cores, *self.out_avals[i].shape)[c]
                for i, name in enumerate(self.out_names)
            })
        return res


_RUNNER_CACHE = {}


def _get_runner(nsteps=T, reps=1):
    key = (nsteps, reps)
    if key not in _RUNNER_CACHE:
        _RUNNER_CACHE[key] = CachedRunner(_get_nc(nsteps, reps), V)
    return _RUNNER_CACHE[key]


def run_cores(inputs, trace=False, nsteps=T):
    """Run the 8-core SPMD kernel; returns (per-core feats [U,B], perf|None)."""
    in_maps = [prep_core(inputs, v) for v in range(V)]
    if trace:
        from concourse.bass_utils import run_bass_kernel_spmd
        res = run_bass_kernel_spmd(_get_nc(nsteps), in_maps,
                                   core_ids=list(range(V)), trace=True)
        return [r["feats"] for r in res.results], res
    try:
        runner = _get_runner(nsteps)
        return [r["feats"] for r in runner.run(in_maps)], None
    except Exception:
        from concourse.bass_utils import run_bass_kernel_spmd
        res = run_bass_kernel_spmd(_get_nc(nsteps), in_maps,
                                   core_ids=list(range(V)))
        return [r["feats"] for r in res.results], res


def kernel(**inputs) -> np.ndarray:
    feats_list, _ = run_cores(inputs)
    feats = np.zeros((B, V * U), dtype=np.float32)
    for v in range(V):
        feats[:, v * U:(v + 1) * U] = feats_list[v].T
    W1 = np.asarray(inputs["W1"], dtype=np.float32)
    b1 = np.asarray(inputs["b1"], dtype=np.float32)
    W2 = np.asarray(inputs["W2"], dtype=np.float32)
    b2 = np.asarray(inputs["b2"], dtype=np.float32)
    h = np.maximum(feats @ W1 + b1, 0.0)
    return (h @ W2 + b2).astype(np.float32)


# revision 17
# speedup vs baseline: 301.0102x; 1.0317x over previous
"""MultiHeadLTC Trainium2 kernel.

V=8 independent LTC heads -> one head per NeuronCore (expert/model parallel).
Per core: B=512, T=64 timesteps x 6 implicit-ODE unfolds, U=64 units.

Device layout (per core):
  state vT [65, 512] SBUF : rows 0..63 = v[u, b], row 64 = ones (bias row).
  Per unfold:
    - 2 init matmuls seed the PSUM accumulator with
        rows 0..63  : cm_t*v + gl*vleak + wnum_sensory   (numerator base)
        rows 64..127: cm_t + gl + eps + wden_sensory     (denominator base)
    - 32 "z" matmuls  z_q[p,b] = sigma_f*(v[i(f),b] - mu_f), f = 128q+p
      (lhsT A_q [65,128] carries sigma-scaled selector rows + bias row)
    - Sigmoid on ScalarE over big PSUM chunks (4/3-bank chunks)
    - 32 accumulating "reduce" matmuls with block-sparse weight maps M_q
      add  sum_i w_p*erev*sig(..)  to numerator rows and
           sum_i w_p*sig(..)      to denominator rows
    - v <- num * reciprocal_approx(den)   (2 DVE ops)
Final: feats[u,b] = v*output_w + output_b  -> DMA out; classifier done on host
(67 MFLOP, 0.003% of total work).
"""

from contextlib import ExitStack

import ml_dtypes
import numpy as np

UNFOLDS, EPS = 6, 1e-8
V, B, T, I, U, H, C = 8, 512, 64, 1, 64, 256, 10
NQ = (U * U) // 128  # 32 z/reduce matmul chunks per unfold


def _softplus(x):
    return np.logaddexp(x.astype(np.float64), 0.0)


def prep_core(inp, v):
    """Host-side precompute of per-core device inputs (all float32)."""
    g = {k: np.asarray(inp[k])[v].astype(np.float64) for k in
         ("gleak", "vleak", "cm", "w", "sigma", "mu", "erev",
          "sensory_w", "sensory_sigma", "sensory_mu", "sensory_erev",
          "input_w", "input_b", "output_w", "output_b")}
    x = np.asarray(inp["x"])[v].astype(np.float32)  # [B, T, I]
    cm_t = _softplus(g["cm"]) * UNFOLDS
    gl = _softplus(g["gleak"])
    w_p = _softplus(g["w"])
    sw_p = _softplus(g["sensory_w"])
    sigma, mu, erev = g["sigma"], g["mu"], g["erev"]
    ssig, smu, serev = (g["sensory_sigma"][0], g["sensory_mu"][0],
                        g["sensory_erev"][0])
    iw, ib = g["input_w"][0], g["input_b"][0]

    f = np.arange(U * U)
    i_f, j_f = f // U, f % U
    sig_f, mu_f = sigma[i_f, j_f], mu[i_f, j_f]
    A = np.zeros((U + 1, U * U))
    A[i_f, f] = sig_f
    A[U, f] = -sig_f * mu_f
    Aall = A.reshape(U + 1, NQ, 128)                  # [65, q, p]

    we = w_p * erev
    M = np.zeros((U * U, 2 * U))
    M[f, j_f] = we[i_f, j_f]
    M[f, U + j_f] = w_p[i_f, j_f]
    Mall = np.ascontiguousarray(
        M.reshape(NQ, 128, 2 * U).transpose(1, 0, 2))  # [p, q, m]

    cmv = np.zeros((U, 128))
    cmv[np.arange(U), np.arange(U)] = cm_t
    ident = np.eye(128)

    Asrow = (ssig * iw)[None, :]                      # [1, U]
    cvec = np.stack([
        sw_p[0] * serev,                              # 0: cne
        sw_p[0],                                      # 1: cnd
        gl * g["vleak"],                              # 2: glv
        cm_t + gl + EPS,                              # 3: cden
        ssig * (ib - smu),                            # 4: sensory ACT bias
        g["output_w"],                                # 5: ow
        g["output_b"],                                # 6: ob
        np.zeros(U),                                  # 7: pad
    ], axis=1)                                        # [U, 8]
    xT = np.ascontiguousarray(x[:, :, 0].T)             # [T, B]

    f32 = np.float32
    bf16 = ml_dtypes.bfloat16
    return dict(xT=xT.astype(f32), Aall=Aall.astype(bf16),
                Mall=Mall.astype(bf16), cmv=cmv.astype(f32),
                ident=ident.astype(f32), Asrow=Asrow.astype(f32),
                cvec=cvec.astype(f32))


def build_nc(nsteps=T, reps=1):
    import concourse.tile as tile
    from concourse import bacc, mybir

    f32 = mybir.dt.float32
    bf16 = mybir.dt.bfloat16
    AF = mybir.ActivationFunctionType
    OP = mybir.AluOpType

    nc = bacc.Bacc("TRN2", target_bir_lowering=False)
    xT_d = nc.dram_tensor("xT", [T, B], f32, kind="ExternalInput")
    Aall_d = nc.dram_tensor("Aall", [U + 1, NQ, 128], bf16,
                            kind="ExternalInput")
    Mall_d = nc.dram_tensor("Mall", [128, NQ, 128], bf16,
                            kind="ExternalInput")
    cmv_d = nc.dram_tensor("cmv", [U, 128], f32, kind="ExternalInput")
    ident_d = nc.dram_tensor("ident", [128, 128], f32, kind="ExternalInput")
    Asrow_d = nc.dram_tensor("Asrow", [1, U], f32, kind="ExternalInput")
    cvec_d = nc.dram_tensor("cvec", [U, 8], f32, kind="ExternalInput")
    feats_d = nc.dram_tensor("feats", [U, B], f32, kind="ExternalOutput")

    with tile.TileContext(nc) as tc, ExitStack() as ctx:
        const = ctx.enter_context(tc.tile_pool(name="const", bufs=1))
        sp = ctx.enter_context(tc.tile_pool(name="sp", bufs=2))
        pz = ctx.enter_context(tc.tile_pool(name="pz", bufs=1, space="PSUM"))

        Aall_sb = const.tile([U + 1, NQ, 128], bf16)
        nc.sync.dma_start(out=Aall_sb, in_=Aall_d[:, :, :])
        Mall_sb = const.tile([128, NQ, 128], bf16)
        nc.sync.dma_start(out=Mall_sb, in_=Mall_d[:, :, :])
        cmv_sb = const.tile([U, 128], f32)
        nc.sync.dma_start(out=cmv_sb, in_=cmv_d[:, :])
        ident_sb = const.tile([128, 128], f32)
        nc.sync.dma_start(out=ident_sb, in_=ident_d[:, :])
        Asrow_sb = const.tile([1, U], f32)
        nc.sync.dma_start(out=Asrow_sb, in_=Asrow_d[:, :])
        cvec_sb = const.tile([U, 8], f32)
        nc.sync.dma_start(out=cvec_sb, in_=cvec_d[:, :])

        vT = const.tile([U + 1, B], f32)
        nc.vector.memset(vT[0:U, :], 0.0)
        nc.vector.memset(vT[U:U + 1, :], 1.0)
        vTb = const.tile([U + 1, B], bf16)   # bf16 shadow for z-matmul rhs
        nc.vector.memset(vTb[0:U, :], 0.0)
        nc.vector.memset(vTb[U:U + 1, :], 1.0)

        HB = B // 2                      # half-batch columns
        CQ = 4                           # q's per ACT chunk per half
        NCH = NQ // CQ                   # 8 chunks per half per unfold
        hs = [slice(0, HB), slice(HB, B)]

        for _rep in range(reps):
          for t in range(nsteps):
            # ---- sensory path (per step) ----
            xrow = sp.tile([1, B], f32, tag="xrow")
            nc.sync.dma_start(out=xrow[:, :], in_=xT_d[t:t + 1, :])
            zs = pz.tile([128, B], f32, tag="z1")
            nc.tensor.matmul(zs[0:U, :], Asrow_sb[:, :], xrow[0:1, :],
                             start=True, stop=True)
            sact = sp.tile([U, B], f32, tag="sact")
            nc.scalar.activation(sact[:, :], zs[0:U, :], AF.Sigmoid,
                                 bias=cvec_sb[:, 4:5], scale=1.0)
            bb = sp.tile([128, B], f32, tag="bb")
            nc.vector.tensor_scalar(bb[0:U, :], sact[:, :],
                                    cvec_sb[:, 0:1], cvec_sb[:, 2:3],
                                    OP.mult, OP.add)
            nc.vector.tensor_scalar(bb[U:128, :], sact[:, :],
                                    cvec_sb[:, 1:2], cvec_sb[:, 3:4],
                                    OP.mult, OP.add)

            for k in range(UNFOLDS):
                acc = [pz.tile([128, HB], f32, tag=f"acc{h}",
                               name=f"acc{h}_{t}_{k}")
                       for h in (0, 1)]
                for h in (0, 1):
                    nc.tensor.matmul(acc[h][:, :], ident_sb[:, :],
                                     bb[:, hs[h]], start=True, stop=False)
                    nc.tensor.matmul(acc[h][:, :], cmv_sb[:, :],
                                     vT[0:U, hs[h]], start=False, stop=False,
                                     skip_group_check=True)
                fd = CQ * HB
                seq = [(h, c) for c in range(NCH) for h in (0, 1)]
                pend = []

                def emit_red(h, c, S):
                    for j in range(CQ):
                        q = c * CQ + j
                        nc.tensor.matmul(acc[h][:, :], Mall_sb[:, q, :],
                                         S[:, j * HB:(j + 1) * HB],
                                         start=False, stop=(q == NQ - 1),
                                         skip_group_check=True)

                for idx, (h, c) in enumerate(seq):
                    slot = idx % 3
                    z = pz.tile([128, fd], f32, tag=f"z{slot}",
                                name=f"z_{t}_{k}_{idx}")
                    for j in range(CQ):
                        q = c * CQ + j
                        nc.tensor.matmul(z[:, j * HB:(j + 1) * HB],
                                         Aall_sb[:, q, :], vTb[:, hs[h]],
                                         start=True, stop=True)
                    S = sp.tile([128, fd], bf16, tag=f"S{slot}", bufs=2,
                                name=f"S_{t}_{k}_{idx}")
                    nc.scalar.activation(S[:, :], z[:, :], AF.Sigmoid)
                    pend.append((h, c, S))
                    if idx >= 2:
                        emit_red(*pend.pop(0))
                for item in pend:
                    emit_red(*item)
                for h in (0, 1):
                    den_cp = sp.tile([U, HB], f32, tag=f"den_cp{h}")
                    nc.vector.tensor_copy(den_cp[:, :], acc[h][U:128, :])
                    rec = sp.tile([U, HB], f32, tag=f"rec{h}")
                    nc.vector.reciprocal_approx_fast(out=rec[:, :],
                                                     in_=den_cp[:, :])
                    nc.vector.tensor_tensor(vT[0:U, hs[h]], acc[h][0:U, :],
                                            rec[:, :], OP.mult)
                    nc.vector.tensor_copy(vTb[0:U, hs[h]], vT[0:U, hs[h]])

        outsb = sp.tile([U, B], f32, tag="outsb")
        nc.vector.tensor_scalar(outsb[:, :], vT[0:U, :],
                                cvec_sb[:, 5:6], cvec_sb[:, 6:7],
                                OP.mult, OP.add)
        nc.sync.dma_start(out=feats_d[:, :], in_=outsb[:, :])
    nc.compile()
    return nc



_NC_CACHE = {}


def _get_nc(nsteps=T, reps=1):
    key = (nsteps, reps)
    if key not in _NC_CACHE:
        _NC_CACHE[key] = build_nc(nsteps, reps)
    return _NC_CACHE[key]


class CachedRunner:
    def __init__(self, nc, n_cores):
        import jax
        from jax.sharding import Mesh, PartitionSpec
        from jax.experimental.shard_map import shard_map
        from concourse import mybir
        from concourse.bass2jax import (_bass_exec_p, install_neuronx_cc_hook,
                                        partition_id_tensor)

        install_neuronx_cc_hook()
        self.nc = nc
        self.n_cores = n_cores
        partition_name = (nc.partition_id_tensor.name
                          if nc.partition_id_tensor else None)
        in_names, out_names, out_avals, zero_outs = [], [], [], []
        for alloc in nc.m.functions[0].allocations:
            if not isinstance(alloc, mybir.MemoryLocationSet):
                continue
            name = alloc.memorylocations[0].name
            if alloc.kind == "ExternalInput":
                if name != partition_name:
                    in_names.append(name)
            elif alloc.kind == "ExternalOutput":
                shape = tuple(alloc.tensor_shape)
                dtype = mybir.dt.np(alloc.dtype)
                out_names.append(name)
                out_avals.append(jax.core.ShapedArray(shape, dtype))
                zero_outs.append(np.zeros(shape, dtype))
        self.in_names, self.out_names = in_names, out_names
        self.out_avals, self.zero_outs = out_avals, zero_outs
        n_params, n_outs = len(in_names), len(out_names)
        self.n_params = n_params
        all_in = list(in_names) + list(out_names)
        if partition_name is not None:
            all_in.append(partition_name)

        def _body(*args):
            operands = list(args)
            if partition_name is not None:
                operands.append(partition_id_tensor())
            return tuple(_bass_exec_p.bind(
                *operands,
                out_avals=tuple(out_avals),
                in_names=tuple(all_in),
                out_names=tuple(out_names),
                lowering_input_output_aliases=(),
                sim_require_finite=True,
                sim_require_nnan=True,
                nc=nc,
            ))

        devices = jax.devices()[:n_cores]
        self.mesh = Mesh(np.asarray(devices), ("core",))
        in_specs = (PartitionSpec("core"),) * (n_params + n_outs)
        out_specs = (PartitionSpec("core"),) * n_outs
        # NOTE: no donation — lets us reuse the same zero buffers across calls.
        self.fn = jax.jit(shard_map(_body, mesh=self.mesh, in_specs=in_specs,
                                    out_specs=out_specs, check_rep=False),
                          keep_unused=True)
        self._jax = jax
        self._zeros_dev = None

    def put_inputs(self, in_maps):
        """Concatenate per-core inputs and move to devices; returns handle."""
        jax = self._jax
        from jax.sharding import NamedSharding, PartitionSpec
        concat_in = [
            np.concatenate([np.asarray(in_maps[c][name])
                            for c in range(self.n_cores)], axis=0)
            for name in self.in_names
        ]
        concat_zeros = [
            np.zeros((self.n_cores * z.shape[0], *z.shape[1:]), z.dtype)
            for z in self.zero_outs
        ]
        sh = NamedSharding(self.mesh, PartitionSpec("core"))
        args = [jax.device_put(a, sh) for a in concat_in + concat_zeros]
        jax.block_until_ready(args)
        return args

    def execute(self, args):
        out = self.fn(*args)
        self._jax.block_until_ready(out)
        return out

    def run(self, in_maps):
        """Full path: transfer + execute + fetch. Returns per-core dicts."""
        args = self.put_inputs(in_maps)
        out_arrs = self.execute(args)
        res = []
        for c in range(self.n_cores):
            res.append({
                name: np.asarray(out_arrs[i]).reshape(
                    self.n_cores, *self.out_avals[i].shape)[c]
                for i, name in enumerate(self.out_names)
            })
        return res


_RUNNER_CACHE = {}


def _get_runner(nsteps=T, reps=1):
    key = (nsteps, reps)
    if key not in _RUNNER_CACHE:
        _RUNNER_CACHE[key] = CachedRunner(_get_nc(nsteps, reps), V)
    return _RUNNER_CACHE[key]


def run_cores(inputs, trace=False, nsteps=T):
    """Run the 8-core SPMD kernel; returns (per-core feats [U,B], perf|None)."""
    in_maps = [prep_core(inputs, v) for v in range(V)]
    if trace:
        from concourse.bass_utils import run_bass_kernel_spmd
        res = run_bass_kernel_spmd(_get_nc(nsteps), in_maps,
                                   core_ids=list(range(V)), trace=True)
        return [r["feats"] for r in res.results], res
    try:
        runner = _get_runner(nsteps)
        return [r["feats"] for r in runner.run(in_maps)], None
    except Exception:
        from concourse.bass_utils import run_bass_kernel_spmd
        res = run_bass_kernel_spmd(_get_nc(nsteps), in_maps,
                                   core_ids=list(range(V)))
        return [r["feats"] for r in res.results], res


def kernel(**inputs) -> np.ndarray:
    feats_list, _ = run_cores(inputs)
    feats = np.zeros((B, V * U), dtype=np.float32)
    for v in range(V):
        feats[:, v * U:(v + 1) * U] = feats_list[v].T
    W1 = np.asarray(inputs["W1"], dtype=np.float32)
    b1 = np.asarray(inputs["b1"], dtype=np.float32)
    W2 = np.asarray(inputs["W2"], dtype=np.float32)
    b2 = np.asarray(inputs["b2"], dtype=np.float32)
    h = np.maximum(feats @ W1 + b1, 0.0)
    return (h @ W2 + b2).astype(np.float32)
